# revision 13
# baseline (speedup 1.0000x reference)
"""GAT (2-layer, single-head) Trainium2 Bass kernel, 8-core SPMD.

Strategy (edge/graph parallelism per the sharding hint):
  - Destination nodes are 1D-sharded: core c owns nodes [c*12500, (c+1)*12500).
  - Edges are routed to the core owning their destination (host bucketing by
    dst), grouped into 128-node dst blocks; per block, edges are bucketed by
    source segment (32k node ranges) so gathers can use int16 indices.
  - Each core computes its slice of the per-node feature table
    feat = [1 | x@W | x@W@al | x@W@ar | pad] ([12544, 64] f32, 256B rows)
    and the full table is AllGathered so every core can gather src rows.
  - Edge phase per dst block: one dma_gather per (block-group, segment) pulls
    feat[src] rows; one dma_gather per block-group pulls er[dst] (from the
    core-local slice); attention logits e = leaky_relu(el[src] + er[dst]),
    ex = exp(e) (softmax shift-invariance: max-subtraction dropped; logits
    are O(1) so fp32 exp is safe); a one-hot segment-reduction matmul per
    128-edge chunk: onehot_ex = (iota == dst_local) * ex built in a single
    two-op tensor_scalar, then PSUM-accumulated out = onehot_ex^T @ [1|feat],
    giving softmax denominator (col 0) and numerator in one chain.
  - out_block = numerator / denom + bias (+ relu between layers); layer-2
    table is produced inline per block (PE transpose + matmul), AllGathered,
    and the edge phase repeats; each core writes its [12500, 32] out slice.
"""

import numpy as np

N = 100000
E = 1600000
F = 128
H = 32
NCORES = 8
NPC = N // NCORES          # nodes per core
P = 128
NB = (NPC + P - 1) // P    # dst blocks per core (98; last block 84 rows)
LB = NPC - (NB - 1) * P    # rows in last block
NPCP = NB * P              # padded nodes per core (table rows per core)
TROWS = NCORES * NPCP      # full table rows
TW = 64                    # table row: [1, feat(32), el, er, pad...] = 256B
ELCOL = 1 + H              # 33
ERCOL = 2 + H              # 34
SEG = 32768                # src segment size (int16 gather indices)
NSEG = (TROWS + SEG - 1) // SEG
GB = 1                     # dst blocks per gather group
NGB = (NB + GB - 1) // GB
CAPC = 8                   # max chunks (1024 indices) per dma_gather op

_cache = {}


def _plan(R):
    """Chunk-column layout shared by host prep and program build.

    R: [NB, NSEG] chunks per (block, segment).
    Returns (CH, groups) where groups[g] = (cg0, Rg, feat_ops, blocks);
    feat_ops[s] = (c0, R_gs); blocks[b] = list of (s, c0, Rbs).
    """
    groups = []
    c = 0
    for g in range(NGB):
        bs = list(range(g * GB, min((g + 1) * GB, NB)))
        cg0 = c
        feat_ops = []
        blocks = {b: [] for b in bs}
        for s in range(NSEG):
            c0 = c
            for b in bs:
                blocks[b].append((s, c, int(R[b, s])))
                c += int(R[b, s])
            feat_ops.append((c0, c - c0))
        groups.append((cg0, c - cg0, feat_ops, blocks))
    return c, groups


def _wrap16(i_op):
    """dma_gather index layout: op-local index i -> (row, col16)."""
    return i_op % 16, i_op // 16


def _host_prep(x, src, dst, W1, al1, ar1, b1, W2, al2, ar2, b2):
    f32, i16 = np.float32, np.int16
    src = np.asarray(src).astype(np.int64)
    dst = np.asarray(dst).astype(np.int64)

    core = dst // NPC
    r = dst % NPC
    b = r // P
    dl = (r % P).astype(f32)
    trow_src = (src // NPC) * NPCP + (src % NPC)
    seg = trow_src // SEG
    bgid = core * NB + b

    order = np.lexsort((seg, bgid))
    s_src_trow = trow_src[order]
    s_seg = seg[order]
    s_bgid = bgid[order]
    s_dl = dl[order]
    s_b = b[order]

    key = s_bgid * NSEG + s_seg
    counts = np.bincount(key, minlength=NCORES * NB * NSEG)
    counts3 = counts.reshape(NCORES, NB, NSEG)
    R = -(-counts3.max(axis=0) // P)          # [NB, NSEG] chunks (may be 0)
    R = np.maximum(R, 1)
    CH, groups = _plan(R)

    # per-edge rank within its (core, block, seg) run
    seg_start = np.concatenate([[0], np.cumsum(counts)])[:-1]
    rank = np.arange(len(order), dtype=np.int64) - seg_start[key]

    # chunk column of each (block, seg): c0 table
    c0_tab = np.zeros((NB, NSEG), np.int64)
    for g, (cg0, Rg, feat_ops, blocks) in enumerate(groups):
        for bb, lst in blocks.items():
            for (s, c0, _Rbs) in lst:
                c0_tab[bb, s] = c0
    cg0_of_b = np.zeros(NB, np.int64)
    for g, (cg0, Rg, feat_ops, blocks) in enumerate(groups):
        for bb in blocks:
            cg0_of_b[bb] = cg0

    col = c0_tab[s_b, s_seg] + rank // P
    p = rank % P

    dstl = np.full((NCORES, P, CH), 200.0, f32)
    flat = s_bgid // NB * (P * CH) + p * CH + col
    dstl.reshape(-1)[flat] = s_dl

    # feat gather indices: [16, CH*8] wrapped, relative to segment base
    fidx = np.zeros((NCORES, 16, CH * 8), i16)
    i_op = (col - c0_tab[s_b, s_seg]) * P + p
    row16, col16 = _wrap16(i_op)
    abscol = c0_tab[s_b, s_seg] * 8 + col16
    fflat = (s_bgid // NB) * (16 * CH * 8) + row16 * (CH * 8) + abscol
    fidx.reshape(-1)[fflat] = (s_src_trow - s_seg * SEG).astype(i16)

    # er gather indices: relative to group window (GB*128 rows)
    eidx = np.zeros((NCORES, 16, CH * 8), i16)
    i_op = (col - cg0_of_b[s_b]) * P + p
    row16, col16 = _wrap16(i_op)
    abscol = cg0_of_b[s_b] * 8 + col16
    eflat = (s_bgid // NB) * (16 * CH * 8) + row16 * (CH * 8) + abscol
    er_local = (s_b - (s_b // GB) * GB) * P + s_dl.astype(np.int64)
    eidx.reshape(-1)[eflat] = er_local.astype(i16)

    fidx = np.tile(fidx, (1, 8, 1))
    eidx = np.tile(eidx, (1, 8, 1))

    def aug(W, al, ar):
        Wa = np.zeros((W.shape[0], TW), f32)
        Wa[:, 1:1 + H] = W
        Wa[:, ELCOL] = W @ al
        Wa[:, ERCOL] = W @ ar
        return Wa

    W1a = aug(np.asarray(W1, f32), np.asarray(al1, f32), np.asarray(ar1, f32))
    W2a = aug(np.asarray(W2, f32), np.asarray(al2, f32), np.asarray(ar2, f32))
    b1r = np.tile(np.asarray(b1, f32)[None, :], (P, 1))
    b2r = np.tile(np.asarray(b2, f32)[None, :], (P, 1))
    iota = np.tile(np.arange(P, dtype=f32)[None, :], (P, 1))

    x = np.asarray(x, f32)
    xsT = np.zeros((NCORES, F, NPCP), f32)
    for cc in range(NCORES):
        xsT[cc, :, :NPC] = x[cc * NPC:(cc + 1) * NPC].T

    in_maps = []
    for cc in range(NCORES):
        in_maps.append({
            "xsT": xsT[cc],
            "W1a": W1a, "W2a": W2a, "b1r": b1r, "b2r": b2r, "iota": iota,
            "fidx": fidx[cc], "eidx": eidx[cc], "dstl": dstl[cc],
        })
    return in_maps, tuple(int(v) for v in R.reshape(-1))


def _build_program(R_key, single=False):
    import concourse.bacc as bacc
    import concourse.mybir as mybir
    import concourse.tile as tile
    from concourse.masks import make_identity

    dt = mybir.dt
    R = np.asarray(R_key, np.int64).reshape(NB, NSEG)
    CH, groups = _plan(R)
    ncores = 1 if single else NCORES

    nc = bacc.Bacc("TRN2", target_bir_lowering=False, debug=False,
                   num_devices=ncores, num_swdge_queues=4)

    xsT = nc.dram_tensor("xsT", [F, NPCP], dt.float32, kind="ExternalInput")
    W1a = nc.dram_tensor("W1a", [F, TW], dt.float32, kind="ExternalInput")
    W2a = nc.dram_tensor("W2a", [H, TW], dt.float32, kind="ExternalInput")
    b1r = nc.dram_tensor("b1r", [P, H], dt.float32, kind="ExternalInput")
    b2r = nc.dram_tensor("b2r", [P, H], dt.float32, kind="ExternalInput")
    iota = nc.dram_tensor("iota", [P, P], dt.float32, kind="ExternalInput")
    fidx = nc.dram_tensor("fidx", [P, CH * 8], dt.int16, kind="ExternalInput")
    eidx = nc.dram_tensor("eidx", [P, CH * 8], dt.int16, kind="ExternalInput")
    dstl = nc.dram_tensor("dstl", [P, CH], dt.float32, kind="ExternalInput")
    out_ext = nc.dram_tensor("out", [NPC, H], dt.float32, kind="ExternalOutput")

    qn_state = [0]

    def qn():
        qn_state[0] = (qn_state[0] + 1) % 4
        return qn_state[0]

    with tile.TileContext(nc) as tc:
        with (
            tc.tile_pool(name="const", bufs=1) as const,
            tc.tile_pool(name="prod", bufs=3) as prod,
            tc.tile_pool(name="gath", bufs=6) as gpool,
            tc.tile_pool(name="erg", bufs=3) as erpool,
            tc.tile_pool(name="edge", bufs=4) as epool,
            tc.tile_pool(name="oh", bufs=6) as ohpool,
            tc.tile_pool(name="epi", bufs=3) as epipool,
            tc.tile_pool(name="ps", bufs=3, space="PSUM") as psum,
            tc.tile_pool(name="pst", bufs=2, space="PSUM") as psumt,
            tc.tile_pool(name="dram", bufs=1, space="DRAM") as dram,
        ):
            iota_sb = const.tile([P, P], dt.float32)
            nc.sync.dma_start(out=iota_sb[:], in_=iota[:])
            W1a_sb = const.tile([F, TW], dt.float32)
            nc.sync.dma_start(out=W1a_sb[:], in_=W1a[:])
            W2a_sb = const.tile([H, TW], dt.float32)
            nc.sync.dma_start(out=W2a_sb[:], in_=W2a[:])
            b1r_sb = const.tile([P, H], dt.float32)
            nc.sync.dma_start(out=b1r_sb[:], in_=b1r[:])
            b2r_sb = const.tile([P, H], dt.float32)
            nc.sync.dma_start(out=b2r_sb[:], in_=b2r[:])
            fidx_sb = const.tile([P, CH * 8], dt.int16)
            nc.sync.dma_start(out=fidx_sb[:], in_=fidx[:])
            eidx_sb = const.tile([P, CH * 8], dt.int16)
            nc.sync.dma_start(out=eidx_sb[:], in_=eidx[:])
            dstl_sb = const.tile([P, CH], dt.float32)
            nc.sync.dma_start(out=dstl_sb[:], in_=dstl[:])
            ident = const.tile([P, P], dt.float32)
            make_identity(nc, ident[:])

            feat1_s = dram.tile([NPCP, TW], dt.float32)
            feat1_f = dram.tile([TROWS, TW], dt.float32, addr_space="Shared")
            feat2_s = dram.tile([NPCP, TW], dt.float32)
            feat2_f = dram.tile([TROWS, TW], dt.float32, addr_space="Shared")

            # ---- layer-1 table production ----
            for b in range(NB):
                xt = prod.tile([F, P], dt.float32, tag="xt")
                nc.sync.dma_start(out=xt[:], in_=xsT[:, b * P:(b + 1) * P])
                pmm = psumt.tile([P, TW], dt.float32, tag="pmm")
                nc.tensor.matmul(out=pmm[:], lhsT=xt[:], rhs=W1a_sb[:],
                                 start=True, stop=True)
                fsb = prod.tile([P, TW], dt.float32, tag="fsb")
                nc.vector.tensor_copy(out=fsb[:, 1:], in_=pmm[:, 1:])
                nc.vector.memset(fsb[:, 0:1], 1.0)
                nc.sync.dma_start(out=feat1_s[b * P:(b + 1) * P, :],
                                  in_=fsb[:])

            def allgather(src_t, dst_t):
                if single:
                    nc.sync.dma_start(out=dst_t[0:NPCP, :], in_=src_t[:])
                else:
                    nc.gpsimd.collective_compute(
                        "AllGather", mybir.AluOpType.bypass,
                        replica_groups=[list(range(NCORES))],
                        ins=[src_t[:]], outs=[dst_t[:]],
                    )

            allgather(feat1_s, feat1_f)

            # ---- edge phase ----
            def edge_phase(feat_f, feat_s, bias_sb, relu, out_writer):
                def emit_gather(tt, in_ap, idx_sb, c0, R):
                    # ucode caps one dma_gather at 1024 indices (8 chunks)
                    tv = tt[:].rearrange("p (r e) -> p r e", e=TW)
                    for off in range(0, R, CAPC):
                        take = min(CAPC, R - off)
                        nc.gpsimd.dma_gather(
                            out_ap=tv[:, off:off + take, :],
                            in_ap=in_ap,
                            idxs_ap=idx_sb[:, (c0 + off) * 8:
                                           (c0 + off + take) * 8],
                            num_idxs=take * P, num_idxs_reg=take * P,
                            elem_size=TW, queue_num=qn(),
                        )

                for g, (cg0, Rg, feat_ops, blocks) in enumerate(groups):
                    erg = erpool.tile([P, Rg * TW], dt.float32, tag="erg")
                    emit_gather(
                        erg,
                        feat_s[g * GB * P:(g * GB + len(blocks)) * P, :],
                        eidx_sb, cg0, Rg)
                    tts = []
                    for s, (c0, Rgs) in enumerate(feat_ops):
                        tt = gpool.tile([P, Rgs * TW], dt.float32,
                                        tag=f"T{s}")
                        seg_lo = s * SEG
                        seg_hi = min(seg_lo + SEG, TROWS)
                        emit_gather(tt, feat_f[seg_lo:seg_hi, :],
                                    fidx_sb, c0, Rgs)
                        tts.append((tt, c0))
                    for b in sorted(blocks):
                        chunks = blocks[b]   # [(s, c0, Rbs)]
                        nch = sum(rr for (_s, _c, rr) in chunks)
                        pacc = psum.tile([P, 1 + H], dt.float32, tag="pacc")
                        done = 0
                        for (s, c0b, Rbs) in chunks:
                            tt, c0op = tts[s]
                            tv = tt[:].rearrange("p (r e) -> p r e", e=TW)
                            ev = erg[:].rearrange("p (r e) -> p r e", e=TW)
                            rb0 = c0b - c0op
                            re0 = c0b - cg0
                            ee = epool.tile([P, Rbs], dt.float32, tag="ee")
                            nc.vector.tensor_tensor(
                                out=ee[:], in0=tv[:, rb0:rb0 + Rbs, ELCOL],
                                in1=ev[:, re0:re0 + Rbs, ERCOL],
                                op=mybir.AluOpType.add)
                            et = epool.tile([P, Rbs], dt.float32, tag="et")
                            nc.vector.tensor_scalar_mul(out=et[:], in0=ee[:],
                                                        scalar1=0.2)
                            nc.vector.tensor_tensor(
                                out=ee[:], in0=ee[:], in1=et[:],
                                op=mybir.AluOpType.max)
                            ex = epool.tile([P, Rbs], dt.float32, tag="ex")
                            nc.scalar.activation(
                                out=ex[:], in_=ee[:],
                                func=mybir.ActivationFunctionType.Exp)
                            for rr in range(Rbs):
                                oh = ohpool.tile([P, P], dt.float32, tag="oh")
                                nc.vector.tensor_scalar(
                                    out=oh[:], in0=iota_sb[:],
                                    scalar1=dstl_sb[:, c0b + rr:c0b + rr + 1],
                                    scalar2=ex[:, rr:rr + 1],
                                    op0=mybir.AluOpType.is_equal,
                                    op1=mybir.AluOpType.mult,
                                )
                                nc.tensor.matmul(
                                    out=pacc[:], lhsT=oh[:],
                                    rhs=tv[:, rb0 + rr, 0:1 + H],
                                    start=(done == 0),
                                    stop=(done == nch - 1),
                                )
                                done += 1
                        den = epipool.tile([P, 1], dt.float32, tag="den")
                        nc.vector.tensor_scalar_add(out=den[:],
                                                    in0=pacc[:, 0:1],
                                                    scalar1=1e-30)
                        rec = epipool.tile([P, 1], dt.float32, tag="rec")
                        nc.vector.reciprocal(out=rec[:], in_=den[:])
                        h = epipool.tile([P, H], dt.float32, tag="h")
                        nc.vector.tensor_scalar_mul(out=h[:], in0=pacc[:, 1:],
                                                    scalar1=rec[:])
                        nc.vector.tensor_tensor(out=h[:], in0=h[:],
                                                in1=bias_sb[:],
                                                op=mybir.AluOpType.add)
                        if relu:
                            nc.scalar.activation(
                                out=h[:], in_=h[:],
                                func=mybir.ActivationFunctionType.Relu)
                        out_writer(b, h)

            def l1_writer(b, h):
                pt = psumt.tile([H, P], dt.float32, tag="pt")
                nc.tensor.transpose(out=pt[:], in_=h[:], identity=ident[:])
                hT = prod.tile([H, P], dt.float32, tag="hT")
                nc.vector.tensor_copy(out=hT[:], in_=pt[:])
                pmm2 = psumt.tile([P, TW], dt.float32, tag="pmm")
                nc.tensor.matmul(out=pmm2[:], lhsT=hT[:], rhs=W2a_sb[:],
                                 start=True, stop=True)
                f2 = prod.tile([P, TW], dt.float32, tag="fsb")
                nc.vector.tensor_copy(out=f2[:, 1:], in_=pmm2[:, 1:])
                nc.vector.memset(f2[:, 0:1], 1.0)
                nc.sync.dma_start(out=feat2_s[b * P:(b + 1) * P, :],
                                  in_=f2[:])

            edge_phase(feat1_f, feat1_s, b1r_sb, True, l1_writer)
            allgather(feat2_s, feat2_f)

            def l2_writer(b, h):
                rows = LB if b == NB - 1 else P
                nc.sync.dma_start(out=out_ext[b * P:b * P + rows, :],
                                  in_=h[:rows, :])

            edge_phase(feat2_f, feat2_s, b2r_sb, False, l2_writer)

    nc.compile()
    return nc


def _get_program(R_key, single=False):
    key = ("prog", R_key, single)
    if key not in _cache:
        _cache[key] = _build_program(R_key, single=single)
    return _cache[key]


def kernel(x, src, dst, W1, al1, ar1, b1, W2, al2, ar2, b2):
    from concourse.bass_utils import run_bass_kernel_spmd

    in_maps, R_key = _host_prep(x, src, dst, W1, al1, ar1, b1,
                                W2, al2, ar2, b2)
    nc = _get_program(R_key)
    res = run_bass_kernel_spmd(nc, in_maps, list(range(NCORES)))
    out = np.concatenate([res.results[c]["out"] for c in range(NCORES)],
                         axis=0)
    return out.astype(np.float32)


# revision 15
# speedup vs baseline: 1977.2285x; 1977.2285x over previous
"""GAT (2-layer, single-head) Trainium2 Bass kernel, 8-core SPMD.

Strategy (edge/graph parallelism per the sharding hint):
  - Destination nodes are 1D-sharded: core c owns nodes [c*12500, (c+1)*12500).
  - Edges are routed to the core owning their destination (host bucketing by
    dst), grouped into 128-node dst blocks; per block, edges are bucketed by
    source segment (32k node ranges) so gathers can use int16 indices.
  - Each core computes its slice of the per-node feature table
    feat = [1 | x@W | x@W@al | x@W@ar | pad] ([12544, 64] f32, 256B rows)
    and the full table is AllGathered so every core can gather src rows.
  - Edge phase per dst block: one dma_gather per (block-group, segment) pulls
    feat[src] rows; one dma_gather per block-group pulls er[dst] (from the
    core-local slice); attention logits e = leaky_relu(el[src] + er[dst]),
    ex = exp(e) (softmax shift-invariance: max-subtraction dropped; logits
    are O(1) so fp32 exp is safe); a one-hot segment-reduction matmul per
    128-edge chunk: onehot_ex = (iota == dst_local) * ex built in a single
    two-op tensor_scalar, then PSUM-accumulated out = onehot_ex^T @ [1|feat],
    giving softmax denominator (col 0) and numerator in one chain.
  - out_block = numerator / denom + bias (+ relu between layers); layer-2
    table is produced inline per block (PE transpose + matmul), AllGathered,
    and the edge phase repeats; each core writes its [12500, 32] out slice.
"""

import numpy as np

N = 100000
E = 1600000
F = 128
H = 32
NCORES = 8
NPC = N // NCORES          # nodes per core
P = 128
NB = (NPC + P - 1) // P    # dst blocks per core (98; last block 84 rows)
LB = NPC - (NB - 1) * P    # rows in last block
NPCP = NB * P              # padded nodes per core (table rows per core)
TROWS = NCORES * NPCP      # full table rows
TW = 64                    # table row: [1, feat(32), el, er, pad...] = 256B
ELCOL = 1 + H              # 33
ERCOL = 2 + H              # 34
SEG = 32768                # src segment size (int16 gather indices)
NSEG = (TROWS + SEG - 1) // SEG
GB = 1                     # dst blocks per gather group
NGB = (NB + GB - 1) // GB
CAPC = 8                   # max chunks (1024 indices) per dma_gather op

_cache = {}


def _plan(R):
    """Chunk-column layout shared by host prep and program build.

    R: [NB, NSEG] chunks per (block, segment).
    Returns (CH, groups) where groups[g] = (cg0, Rg, feat_ops, blocks);
    feat_ops[s] = (c0, R_gs); blocks[b] = list of (s, c0, Rbs).
    """
    groups = []
    c = 0
    for g in range(NGB):
        bs = list(range(g * GB, min((g + 1) * GB, NB)))
        cg0 = c
        feat_ops = []
        blocks = {b: [] for b in bs}
        for s in range(NSEG):
            c0 = c
            for b in bs:
                blocks[b].append((s, c, int(R[b, s])))
                c += int(R[b, s])
            feat_ops.append((c0, c - c0))
        groups.append((cg0, c - cg0, feat_ops, blocks))
    return c, groups


def _wrap16(i_op):
    """dma_gather index layout: op-local index i -> (row, col16)."""
    return i_op % 16, i_op // 16


def _host_prep(x, src, dst, W1, al1, ar1, b1, W2, al2, ar2, b2):
    f32, i16 = np.float32, np.int16
    src = np.asarray(src).astype(np.int64)
    dst = np.asarray(dst).astype(np.int64)

    core = dst // NPC
    r = dst % NPC
    b = r // P
    dl = (r % P).astype(f32)
    trow_src = (src // NPC) * NPCP + (src % NPC)
    seg = trow_src // SEG
    bgid = core * NB + b

    order = np.lexsort((seg, bgid))
    s_src_trow = trow_src[order]
    s_seg = seg[order]
    s_bgid = bgid[order]
    s_dl = dl[order]
    s_b = b[order]

    key = s_bgid * NSEG + s_seg
    counts = np.bincount(key, minlength=NCORES * NB * NSEG)
    counts3 = counts.reshape(NCORES, NB, NSEG)
    R = -(-counts3.max(axis=0) // P)          # [NB, NSEG] chunks (may be 0)
    R = np.maximum(R, 1)
    CH, groups = _plan(R)

    # per-edge rank within its (core, block, seg) run
    seg_start = np.concatenate([[0], np.cumsum(counts)])[:-1]
    rank = np.arange(len(order), dtype=np.int64) - seg_start[key]

    # chunk column of each (block, seg): c0 table
    c0_tab = np.zeros((NB, NSEG), np.int64)
    for g, (cg0, Rg, feat_ops, blocks) in enumerate(groups):
        for bb, lst in blocks.items():
            for (s, c0, _Rbs) in lst:
                c0_tab[bb, s] = c0
    cg0_of_b = np.zeros(NB, np.int64)
    for g, (cg0, Rg, feat_ops, blocks) in enumerate(groups):
        for bb in blocks:
            cg0_of_b[bb] = cg0

    col = c0_tab[s_b, s_seg] + rank // P
    p = rank % P

    dstl = np.full((NCORES, P, CH), 200.0, f32)
    flat = s_bgid // NB * (P * CH) + p * CH + col
    dstl.reshape(-1)[flat] = s_dl

    # feat gather indices: [16, CH*8] wrapped, relative to segment base
    fidx = np.zeros((NCORES, 16, CH * 8), i16)
    i_op = (col - c0_tab[s_b, s_seg]) * P + p
    row16, col16 = _wrap16(i_op)
    abscol = c0_tab[s_b, s_seg] * 8 + col16
    fflat = (s_bgid // NB) * (16 * CH * 8) + row16 * (CH * 8) + abscol
    fidx.reshape(-1)[fflat] = (s_src_trow - s_seg * SEG).astype(i16)

    # er gather indices: relative to group window (GB*128 rows)
    eidx = np.zeros((NCORES, 16, CH * 8), i16)
    i_op = (col - cg0_of_b[s_b]) * P + p
    row16, col16 = _wrap16(i_op)
    abscol = cg0_of_b[s_b] * 8 + col16
    eflat = (s_bgid // NB) * (16 * CH * 8) + row16 * (CH * 8) + abscol
    er_local = (s_b - (s_b // GB) * GB) * P + s_dl.astype(np.int64)
    eidx.reshape(-1)[eflat] = er_local.astype(i16)

    fidx = np.tile(fidx, (1, 8, 1))
    eidx = np.tile(eidx, (1, 8, 1))

    def aug(W, al, ar):
        Wa = np.zeros((W.shape[0], TW), f32)
        Wa[:, 1:1 + H] = W
        Wa[:, ELCOL] = W @ al
        Wa[:, ERCOL] = W @ ar
        return Wa

    W1a = aug(np.asarray(W1, f32), np.asarray(al1, f32), np.asarray(ar1, f32))
    W2a = aug(np.asarray(W2, f32), np.asarray(al2, f32), np.asarray(ar2, f32))
    b1r = np.tile(np.asarray(b1, f32)[None, :], (P, 1))
    b2r = np.tile(np.asarray(b2, f32)[None, :], (P, 1))
    iota = np.tile(np.arange(P, dtype=f32)[None, :], (P, 1))

    x = np.asarray(x, f32)
    xsT = np.zeros((NCORES, F, NPCP), f32)
    for cc in range(NCORES):
        xsT[cc, :, :NPC] = x[cc * NPC:(cc + 1) * NPC].T

    in_maps = []
    for cc in range(NCORES):
        in_maps.append({
            "xsT": xsT[cc],
            "W1a": W1a, "W2a": W2a, "b1r": b1r, "b2r": b2r, "iota": iota,
            "fidx": fidx[cc], "eidx": eidx[cc], "dstl": dstl[cc],
        })
    return in_maps, tuple(int(v) for v in R.reshape(-1))


def _build_program(R_key, single=False):
    import concourse.bacc as bacc
    import concourse.mybir as mybir
    import concourse.tile as tile
    from concourse.masks import make_identity

    dt = mybir.dt
    R = np.asarray(R_key, np.int64).reshape(NB, NSEG)
    CH, groups = _plan(R)
    ncores = 1 if single else NCORES

    nc = bacc.Bacc("TRN2", target_bir_lowering=False, debug=False,
                   num_devices=ncores, num_swdge_queues=4)

    xsT = nc.dram_tensor("xsT", [F, NPCP], dt.float32, kind="ExternalInput")
    W1a = nc.dram_tensor("W1a", [F, TW], dt.float32, kind="ExternalInput")
    W2a = nc.dram_tensor("W2a", [H, TW], dt.float32, kind="ExternalInput")
    b1r = nc.dram_tensor("b1r", [P, H], dt.float32, kind="ExternalInput")
    b2r = nc.dram_tensor("b2r", [P, H], dt.float32, kind="ExternalInput")
    iota = nc.dram_tensor("iota", [P, P], dt.float32, kind="ExternalInput")
    fidx = nc.dram_tensor("fidx", [P, CH * 8], dt.int16, kind="ExternalInput")
    eidx = nc.dram_tensor("eidx", [P, CH * 8], dt.int16, kind="ExternalInput")
    dstl = nc.dram_tensor("dstl", [P, CH], dt.float32, kind="ExternalInput")
    out_ext = nc.dram_tensor("out", [NPC, H], dt.float32, kind="ExternalOutput")

    qn_state = [0]

    def qn():
        qn_state[0] = (qn_state[0] + 1) % 4
        return qn_state[0]

    with tile.TileContext(nc) as tc:
        with (
            tc.tile_pool(name="const", bufs=1) as const,
            tc.tile_pool(name="prod", bufs=3) as prod,
            tc.tile_pool(name="gath", bufs=6) as gpool,
            tc.tile_pool(name="erg", bufs=3) as erpool,
            tc.tile_pool(name="edge", bufs=4) as epool,
            tc.tile_pool(name="oh", bufs=6) as ohpool,
            tc.tile_pool(name="epi", bufs=3) as epipool,
            tc.tile_pool(name="ps", bufs=3, space="PSUM") as psum,
            tc.tile_pool(name="pst", bufs=2, space="PSUM") as psumt,
            tc.tile_pool(name="dram", bufs=1, space="DRAM") as dram,
        ):
            iota_sb = const.tile([P, P], dt.float32)
            nc.sync.dma_start(out=iota_sb[:], in_=iota[:])
            W1a_sb = const.tile([F, TW], dt.float32)
            nc.sync.dma_start(out=W1a_sb[:], in_=W1a[:])
            W2a_sb = const.tile([H, TW], dt.float32)
            nc.sync.dma_start(out=W2a_sb[:], in_=W2a[:])
            b1r_sb = const.tile([P, H], dt.float32)
            nc.sync.dma_start(out=b1r_sb[:], in_=b1r[:])
            b2r_sb = const.tile([P, H], dt.float32)
            nc.sync.dma_start(out=b2r_sb[:], in_=b2r[:])
            fidx_sb = const.tile([P, CH * 8], dt.int16)
            nc.sync.dma_start(out=fidx_sb[:], in_=fidx[:])
            eidx_sb = const.tile([P, CH * 8], dt.int16)
            nc.sync.dma_start(out=eidx_sb[:], in_=eidx[:])
            dstl_sb = const.tile([P, CH], dt.float32)
            nc.sync.dma_start(out=dstl_sb[:], in_=dstl[:])
            ident = const.tile([P, P], dt.float32)
            make_identity(nc, ident[:])

            feat1_s = dram.tile([NPCP, TW], dt.float32)
            feat1_f = dram.tile([TROWS, TW], dt.float32, addr_space="Shared")
            feat2_s = dram.tile([NPCP, TW], dt.float32)
            feat2_f = dram.tile([TROWS, TW], dt.float32, addr_space="Shared")

            # ---- layer-1 table production ----
            for b in range(NB):
                xt = prod.tile([F, P], dt.float32, tag="xt")
                nc.sync.dma_start(out=xt[:], in_=xsT[:, b * P:(b + 1) * P])
                pmm = psumt.tile([P, TW], dt.float32, tag="pmm")
                nc.tensor.matmul(out=pmm[:], lhsT=xt[:], rhs=W1a_sb[:],
                                 start=True, stop=True)
                fsb = prod.tile([P, TW], dt.float32, tag="fsb")
                nc.vector.tensor_copy(out=fsb[:, 1:], in_=pmm[:, 1:])
                nc.vector.memset(fsb[:, 0:1], 1.0)
                nc.sync.dma_start(out=feat1_s[b * P:(b + 1) * P, :],
                                  in_=fsb[:])

            def allgather(src_t, dst_t):
                if single:
                    nc.sync.dma_start(out=dst_t[0:NPCP, :], in_=src_t[:])
                else:
                    nc.gpsimd.collective_compute(
                        "AllGather", mybir.AluOpType.bypass,
                        replica_groups=[list(range(NCORES))],
                        ins=[src_t[:]], outs=[dst_t[:]],
                    )

            allgather(feat1_s, feat1_f)

            # ---- edge phase ----
            def edge_phase(feat_f, feat_s, bias_sb, relu, out_writer):
                def emit_gather(tt, in_ap, idx_sb, c0, R):
                    # ucode caps one dma_gather at 1024 indices (8 chunks)
                    tv = tt[:].rearrange("p (r e) -> p r e", e=TW)
                    for off in range(0, R, CAPC):
                        take = min(CAPC, R - off)
                        nc.gpsimd.dma_gather(
                            out_ap=tv[:, off:off + take, :],
                            in_ap=in_ap,
                            idxs_ap=idx_sb[:, (c0 + off) * 8:
                                           (c0 + off + take) * 8],
                            num_idxs=take * P, num_idxs_reg=take * P,
                            elem_size=TW, queue_num=qn(),
                        )

                for g, (cg0, Rg, feat_ops, blocks) in enumerate(groups):
                    erg = erpool.tile([P, Rg * TW], dt.float32, tag="erg")
                    emit_gather(
                        erg,
                        feat_s[g * GB * P:(g * GB + len(blocks)) * P, :],
                        eidx_sb, cg0, Rg)
                    tts = []
                    for s, (c0, Rgs) in enumerate(feat_ops):
                        tt = gpool.tile([P, Rgs * TW], dt.float32,
                                        tag=f"T{s}")
                        seg_lo = s * SEG
                        seg_hi = min(seg_lo + SEG, TROWS)
                        emit_gather(tt, feat_f[seg_lo:seg_hi, :],
                                    fidx_sb, c0, Rgs)
                        tts.append((tt, c0))
                    for b in sorted(blocks):
                        chunks = blocks[b]   # [(s, c0, Rbs)]
                        nch = sum(rr for (_s, _c, rr) in chunks)
                        pacc = psum.tile([P, 1 + H], dt.float32, tag="pacc")
                        done = 0
                        for (s, c0b, Rbs) in chunks:
                            tt, c0op = tts[s]
                            tv = tt[:].rearrange("p (r e) -> p r e", e=TW)
                            ev = erg[:].rearrange("p (r e) -> p r e", e=TW)
                            rb0 = c0b - c0op
                            re0 = c0b - cg0
                            ee = epool.tile([P, Rbs], dt.float32, tag="ee")
                            nc.vector.tensor_tensor(
                                out=ee[:], in0=tv[:, rb0:rb0 + Rbs, ELCOL],
                                in1=ev[:, re0:re0 + Rbs, ERCOL],
                                op=mybir.AluOpType.add)
                            et = epool.tile([P, Rbs], dt.float32, tag="et")
                            nc.vector.tensor_scalar_mul(out=et[:], in0=ee[:],
                                                        scalar1=0.2)
                            nc.vector.tensor_tensor(
                                out=ee[:], in0=ee[:], in1=et[:],
                                op=mybir.AluOpType.max)
                            ex = epool.tile([P, Rbs], dt.float32, tag="ex")
                            nc.scalar.activation(
                                out=ex[:], in_=ee[:],
                                func=mybir.ActivationFunctionType.Exp)
                            for rr in range(Rbs):
                                oh = ohpool.tile([P, P], dt.float32, tag="oh")
                                nc.vector.tensor_scalar(
                                    out=oh[:], in0=iota_sb[:],
                                    scalar1=dstl_sb[:, c0b + rr:c0b + rr + 1],
                                    scalar2=ex[:, rr:rr + 1],
                                    op0=mybir.AluOpType.is_equal,
                                    op1=mybir.AluOpType.mult,
                                )
                                nc.tensor.matmul(
                                    out=pacc[:], lhsT=oh[:],
                                    rhs=tv[:, rb0 + rr, 0:1 + H],
                                    start=(done == 0),
                                    stop=(done == nch - 1),
                                )
                                done += 1
                        den = epipool.tile([P, 1], dt.float32, tag="den")
                        nc.vector.tensor_scalar_add(out=den[:],
                                                    in0=pacc[:, 0:1],
                                                    scalar1=1e-30)
                        rec = epipool.tile([P, 1], dt.float32, tag="rec")
                        nc.vector.reciprocal(out=rec[:], in_=den[:])
                        h = epipool.tile([P, H], dt.float32, tag="h")
                        nc.vector.tensor_scalar_mul(out=h[:], in0=pacc[:, 1:],
                                                    scalar1=rec[:])
                        nc.vector.tensor_tensor(out=h[:], in0=h[:],
                                                in1=bias_sb[:],
                                                op=mybir.AluOpType.add)
                        if relu:
                            nc.scalar.activation(
                                out=h[:], in_=h[:],
                                func=mybir.ActivationFunctionType.Relu)
                        out_writer(b, h)

            def l1_writer(b, h):
                pt = psumt.tile([H, P], dt.float32, tag="pt")
                nc.tensor.transpose(out=pt[:], in_=h[:], identity=ident[:])
                hT = prod.tile([H, P], dt.float32, tag="hT")
                nc.vector.tensor_copy(out=hT[:], in_=pt[:])
                pmm2 = psumt.tile([P, TW], dt.float32, tag="pmm")
                nc.tensor.matmul(out=pmm2[:], lhsT=hT[:], rhs=W2a_sb[:],
                                 start=True, stop=True)
                f2 = prod.tile([P, TW], dt.float32, tag="fsb")
                nc.vector.tensor_copy(out=f2[:, 1:], in_=pmm2[:, 1:])
                nc.vector.memset(f2[:, 0:1], 1.0)
                nc.sync.dma_start(out=feat2_s[b * P:(b + 1) * P, :],
                                  in_=f2[:])

            edge_phase(feat1_f, feat1_s, b1r_sb, True, l1_writer)
            allgather(feat2_s, feat2_f)

            def l2_writer(b, h):
                rows = LB if b == NB - 1 else P
                nc.sync.dma_start(out=out_ext[b * P:b * P + rows, :],
                                  in_=h[:rows, :])

            edge_phase(feat2_f, feat2_s, b2r_sb, False, l2_writer)

    nc.compile()
    return nc


def _get_program(R_key, single=False):
    key = ("prog", R_key, single)
    if key not in _cache:
        _cache[key] = _build_program(R_key, single=single)
    return _cache[key]


def kernel(x, src, dst, W1, al1, ar1, b1, W2, al2, ar2, b2):
    from concourse.bass_utils import run_bass_kernel_spmd

    in_maps, R_key = _host_prep(x, src, dst, W1, al1, ar1, b1,
                                W2, al2, ar2, b2)
    nc = _get_program(R_key)
    res = run_bass_kernel_spmd(nc, in_maps, list(range(NCORES)))
    out = np.concatenate([res.results[c]["out"] for c in range(NCORES)],
                         axis=0)
    return out.astype(np.float32)


# revision 24
# speedup vs baseline: 2030.5195x; 1.0270x over previous
"""GAT (2-layer, single-head) Trainium2 Bass kernel, 8-core SPMD.

Strategy (edge/graph parallelism per the sharding hint):
  - Destination nodes are 1D-sharded: core c owns nodes [c*12500, (c+1)*12500).
  - Edges are routed to the core owning their destination (host bucketing by
    dst), grouped into 128-node dst blocks; per block, edges are bucketed by
    source segment (32k node ranges) so gathers can use int16 indices.
  - Each core computes its slice of the per-node feature table
    feat = [1 | x@W | x@W@al | x@W@ar | pad] ([12544, 64] f32, 256B rows)
    and the full table is AllGathered so every core can gather src rows.
  - Edge phase per dst block: one dma_gather per (block-group, segment) pulls
    feat[src] rows; one dma_gather per block-group pulls er[dst] (from the
    core-local slice); attention logits e = leaky_relu(el[src] + er[dst]),
    ex = exp(e) (softmax shift-invariance: max-subtraction dropped; logits
    are O(1) so fp32 exp is safe); a one-hot segment-reduction matmul per
    128-edge chunk: onehot_ex = (iota == dst_local) * ex built in a single
    two-op tensor_scalar, then PSUM-accumulated out = onehot_ex^T @ [1|feat],
    giving softmax denominator (col 0) and numerator in one chain.
  - out_block = numerator / denom + bias (+ relu between layers); layer-2
    table is produced inline per block (PE transpose + matmul), AllGathered,
    and the edge phase repeats; each core writes its [12500, 32] out slice.
"""

import numpy as np

N = 100000
E = 1600000
F = 128
H = 32
NCORES = 8
NPC = N // NCORES          # nodes per core
P = 128
NB = (NPC + P - 1) // P    # dst blocks per core (98; last block 84 rows)
LB = NPC - (NB - 1) * P    # rows in last block
NPCP = NB * P              # padded nodes per core (table rows per core)
TROWS = NCORES * NPCP      # full table rows
TW = 64                    # table row: [1, feat(32), el, er, pad...] = 256B
ELCOL = 1 + H              # 33
ERCOL = 2 + H              # 34
SEG = 32768                # src segment size (int16 gather indices)
NSEG = (TROWS + SEG - 1) // SEG
GB = 1                     # dst blocks per gather group
NGB = (NB + GB - 1) // GB
CAPC = 8                   # max chunks (1024 indices) per dma_gather op

_cache = {}


def _plan(R):
    """Chunk-column layout shared by host prep and program build.

    R: [NB, NSEG] chunks per (block, segment).
    Returns (CH, groups) where groups[g] = (cg0, Rg, feat_ops, blocks);
    feat_ops[s] = (c0, R_gs); blocks[b] = list of (s, c0, Rbs).
    """
    groups = []
    c = 0
    for g in range(NGB):
        bs = list(range(g * GB, min((g + 1) * GB, NB)))
        cg0 = c
        feat_ops = []
        blocks = {b: [] for b in bs}
        for s in range(NSEG):
            c0 = c
            for b in bs:
                blocks[b].append((s, c, int(R[b, s])))
                c += int(R[b, s])
            feat_ops.append((c0, c - c0))
        groups.append((cg0, c - cg0, feat_ops, blocks))
    return c, groups


def _wrap16(i_op):
    """dma_gather index layout: op-local index i -> (row, col16)."""
    return i_op % 16, i_op // 16


def _host_prep(x, src, dst, W1, al1, ar1, b1, W2, al2, ar2, b2):
    f32, i16 = np.float32, np.int16
    src = np.asarray(src).astype(np.int64)
    dst = np.asarray(dst).astype(np.int64)

    core = dst // NPC
    r = dst % NPC
    b = r // P
    dl = (r % P).astype(f32)
    trow_src = (src // NPC) * NPCP + (src % NPC)
    seg = trow_src // SEG
    bgid = core * NB + b

    order = np.lexsort((seg, bgid))
    s_src_trow = trow_src[order]
    s_seg = seg[order]
    s_bgid = bgid[order]
    s_dl = dl[order]
    s_b = b[order]

    key = s_bgid * NSEG + s_seg
    counts = np.bincount(key, minlength=NCORES * NB * NSEG)
    counts3 = counts.reshape(NCORES, NB, NSEG)
    R = -(-counts3.max(axis=0) // P)          # [NB, NSEG] chunks (may be 0)
    R = np.maximum(R, 1)
    CH, groups = _plan(R)

    # per-edge rank within its (core, block, seg) run
    seg_start = np.concatenate([[0], np.cumsum(counts)])[:-1]
    rank = np.arange(len(order), dtype=np.int64) - seg_start[key]

    # chunk column of each (block, seg): c0 table
    c0_tab = np.zeros((NB, NSEG), np.int64)
    for g, (cg0, Rg, feat_ops, blocks) in enumerate(groups):
        for bb, lst in blocks.items():
            for (s, c0, _Rbs) in lst:
                c0_tab[bb, s] = c0
    cg0_of_b = np.zeros(NB, np.int64)
    for g, (cg0, Rg, feat_ops, blocks) in enumerate(groups):
        for bb in blocks:
            cg0_of_b[bb] = cg0

    col = c0_tab[s_b, s_seg] + rank // P
    p = rank % P

    dstl = np.full((NCORES, P, CH), 200.0, f32)
    flat = s_bgid // NB * (P * CH) + p * CH + col
    dstl.reshape(-1)[flat] = s_dl

    # feat gather indices: [16, CH*8] wrapped, relative to segment base
    fidx = np.zeros((NCORES, 16, CH * 8), i16)
    i_op = (col - c0_tab[s_b, s_seg]) * P + p
    row16, col16 = _wrap16(i_op)
    abscol = c0_tab[s_b, s_seg] * 8 + col16
    fflat = (s_bgid // NB) * (16 * CH * 8) + row16 * (CH * 8) + abscol
    fidx.reshape(-1)[fflat] = (s_src_trow - s_seg * SEG).astype(i16)

    # er gather indices: relative to group window (GB*128 rows)
    eidx = np.zeros((NCORES, 16, CH * 8), i16)
    i_op = (col - cg0_of_b[s_b]) * P + p
    row16, col16 = _wrap16(i_op)
    abscol = cg0_of_b[s_b] * 8 + col16
    eflat = (s_bgid // NB) * (16 * CH * 8) + row16 * (CH * 8) + abscol
    er_local = (s_b - (s_b // GB) * GB) * P + s_dl.astype(np.int64)
    eidx.reshape(-1)[eflat] = er_local.astype(i16)

    fidx = np.tile(fidx, (1, 8, 1))
    eidx = np.tile(eidx, (1, 8, 1))

    def aug(W, al, ar):
        Wa = np.zeros((W.shape[0], TW), f32)
        Wa[:, 1:1 + H] = W
        Wa[:, ELCOL] = W @ al
        Wa[:, ERCOL] = W @ ar
        return Wa

    W1a = aug(np.asarray(W1, f32), np.asarray(al1, f32), np.asarray(ar1, f32))
    W2a = aug(np.asarray(W2, f32), np.asarray(al2, f32), np.asarray(ar2, f32))
    b1r = np.tile(np.asarray(b1, f32)[None, :], (P, 1))
    b2r = np.tile(np.asarray(b2, f32)[None, :], (P, 1))
    iota = np.tile(np.arange(P, dtype=f32)[None, :], (P, 1))

    x = np.asarray(x, f32)
    xsT = np.zeros((NCORES, F, NPCP), f32)
    for cc in range(NCORES):
        xsT[cc, :, :NPC] = x[cc * NPC:(cc + 1) * NPC].T

    in_maps = []
    for cc in range(NCORES):
        in_maps.append({
            "xsT": xsT[cc],
            "W1a": W1a, "W2a": W2a, "b1r": b1r, "b2r": b2r, "iota": iota,
            "fidx": fidx[cc], "eidx": eidx[cc], "dstl": dstl[cc],
        })
    return in_maps, tuple(int(v) for v in R.reshape(-1))


def _build_program(R_key, single=False):
    import concourse.bacc as bacc
    import concourse.mybir as mybir
    import concourse.tile as tile
    from concourse.masks import make_identity

    dt = mybir.dt
    R = np.asarray(R_key, np.int64).reshape(NB, NSEG)
    CH, groups = _plan(R)
    ncores = 1 if single else NCORES

    nc = bacc.Bacc("TRN2", target_bir_lowering=False, debug=False,
                   num_devices=ncores, num_swdge_queues=4)

    xsT = nc.dram_tensor("xsT", [F, NPCP], dt.float32, kind="ExternalInput")
    W1a = nc.dram_tensor("W1a", [F, TW], dt.float32, kind="ExternalInput")
    W2a = nc.dram_tensor("W2a", [H, TW], dt.float32, kind="ExternalInput")
    b1r = nc.dram_tensor("b1r", [P, H], dt.float32, kind="ExternalInput")
    b2r = nc.dram_tensor("b2r", [P, H], dt.float32, kind="ExternalInput")
    iota = nc.dram_tensor("iota", [P, P], dt.float32, kind="ExternalInput")
    fidx = nc.dram_tensor("fidx", [P, CH * 8], dt.int16, kind="ExternalInput")
    eidx = nc.dram_tensor("eidx", [P, CH * 8], dt.int16, kind="ExternalInput")
    dstl = nc.dram_tensor("dstl", [P, CH], dt.float32, kind="ExternalInput")
    out_ext = nc.dram_tensor("out", [NPC, H], dt.float32, kind="ExternalOutput")

    qn_state = [0]

    def qn():
        qn_state[0] = (qn_state[0] + 1) % 4
        return qn_state[0]

    with tile.TileContext(nc) as tc:
        with (
            tc.tile_pool(name="const", bufs=1) as const,
            tc.tile_pool(name="prod", bufs=3) as prod,
            tc.tile_pool(name="gath", bufs=16) as gpool,
            tc.tile_pool(name="erg", bufs=8) as erpool,
            tc.tile_pool(name="edge", bufs=6) as epool,
            tc.tile_pool(name="oh", bufs=8) as ohpool,
            tc.tile_pool(name="epi", bufs=3) as epipool,
            tc.tile_pool(name="ps", bufs=4, space="PSUM") as psum,
            tc.tile_pool(name="pst", bufs=2, space="PSUM") as psumt,
            tc.tile_pool(name="dram", bufs=1, space="DRAM") as dram,
        ):
            iota_sb = const.tile([P, P], dt.float32)
            nc.sync.dma_start(out=iota_sb[:], in_=iota[:])
            W1a_sb = const.tile([F, TW], dt.float32)
            nc.sync.dma_start(out=W1a_sb[:], in_=W1a[:])
            W2a_sb = const.tile([H, TW], dt.float32)
            nc.sync.dma_start(out=W2a_sb[:], in_=W2a[:])
            b1r_sb = const.tile([P, H], dt.float32)
            nc.sync.dma_start(out=b1r_sb[:], in_=b1r[:])
            b2r_sb = const.tile([P, H], dt.float32)
            nc.sync.dma_start(out=b2r_sb[:], in_=b2r[:])
            fidx_sb = const.tile([P, CH * 8], dt.int16)
            nc.sync.dma_start(out=fidx_sb[:], in_=fidx[:])
            eidx_sb = const.tile([P, CH * 8], dt.int16)
            nc.sync.dma_start(out=eidx_sb[:], in_=eidx[:])
            dstl_sb = const.tile([P, CH], dt.float32)
            nc.sync.dma_start(out=dstl_sb[:], in_=dstl[:])
            ident = const.tile([P, P], dt.float32)
            make_identity(nc, ident[:])

            feat1_s = dram.tile([NPCP, TW], dt.float32)
            feat1_f = dram.tile([TROWS, TW], dt.float32, addr_space="Shared")
            feat2_s = dram.tile([NPCP, TW], dt.float32)
            feat2_f = dram.tile([TROWS, TW], dt.float32, addr_space="Shared")

            # ---- layer-1 table production ----
            for b in range(NB):
                xt = prod.tile([F, P], dt.float32, tag="xt")
                nc.sync.dma_start(out=xt[:], in_=xsT[:, b * P:(b + 1) * P])
                pmm = psumt.tile([P, TW], dt.float32, tag="pmm")
                nc.tensor.matmul(out=pmm[:], lhsT=xt[:], rhs=W1a_sb[:],
                                 start=True, stop=True)
                fsb = prod.tile([P, TW], dt.float32, tag="fsb")
                nc.vector.tensor_copy(out=fsb[:, 1:], in_=pmm[:, 1:])
                nc.vector.memset(fsb[:, 0:1], 1.0)
                nc.sync.dma_start(out=feat1_s[b * P:(b + 1) * P, :],
                                  in_=fsb[:])

            def allgather(src_t, dst_t):
                if single:
                    nc.sync.dma_start(out=dst_t[0:NPCP, :], in_=src_t[:])
                else:
                    nc.gpsimd.collective_compute(
                        "AllGather", mybir.AluOpType.bypass,
                        replica_groups=[list(range(NCORES))],
                        ins=[src_t[:]], outs=[dst_t[:]],
                    )

            allgather(feat1_s, feat1_f)

            # ---- edge phase ----
            def edge_phase(feat_f, feat_s, bias_sb, relu, out_writer):
                def emit_gather(tt, in_ap, idx_sb, c0, R):
                    # ucode caps one dma_gather at 1024 indices (8 chunks)
                    tv = tt[:].rearrange("p (r e) -> p r e", e=TW)
                    for off in range(0, R, CAPC):
                        take = min(CAPC, R - off)
                        nc.gpsimd.dma_gather(
                            out_ap=tv[:, off:off + take, :],
                            in_ap=in_ap,
                            idxs_ap=idx_sb[:, (c0 + off) * 8:
                                           (c0 + off + take) * 8],
                            num_idxs=take * P, num_idxs_reg=take * P,
                            elem_size=TW, queue_num=qn(),
                        )

                for g, (cg0, Rg, feat_ops, blocks) in enumerate(groups):
                    erg = erpool.tile([P, Rg * TW], dt.float32, tag="erg")
                    emit_gather(
                        erg,
                        feat_s[g * GB * P:(g * GB + len(blocks)) * P, :],
                        eidx_sb, cg0, Rg)
                    tts = []
                    for s, (c0, Rgs) in enumerate(feat_ops):
                        tt = gpool.tile([P, Rgs * TW], dt.float32,
                                        tag=f"T{s}")
                        seg_lo = s * SEG
                        seg_hi = min(seg_lo + SEG, TROWS)
                        emit_gather(tt, feat_f[seg_lo:seg_hi, :],
                                    fidx_sb, c0, Rgs)
                        tts.append((tt, c0))
                    for b in sorted(blocks):
                        chunks = blocks[b]   # [(s, c0, Rbs)]
                        nch = sum(rr for (_s, _c, rr) in chunks)
                        pacc = psum.tile([P, 1 + H], dt.float32, tag="pacc")
                        done = 0
                        for (s, c0b, Rbs) in chunks:
                            tt, c0op = tts[s]
                            tv = tt[:].rearrange("p (r e) -> p r e", e=TW)
                            ev = erg[:].rearrange("p (r e) -> p r e", e=TW)
                            rb0 = c0b - c0op
                            re0 = c0b - cg0
                            ee = epool.tile([P, Rbs], dt.float32, tag="ee")
                            nc.vector.tensor_tensor(
                                out=ee[:], in0=tv[:, rb0:rb0 + Rbs, ELCOL],
                                in1=ev[:, re0:re0 + Rbs, ERCOL],
                                op=mybir.AluOpType.add)
                            et = epool.tile([P, Rbs], dt.float32, tag="et")
                            nc.vector.tensor_scalar_mul(out=et[:], in0=ee[:],
                                                        scalar1=0.2)
                            nc.vector.tensor_tensor(
                                out=ee[:], in0=ee[:], in1=et[:],
                                op=mybir.AluOpType.max)
                            ex = epool.tile([P, Rbs], dt.float32, tag="ex")
                            nc.scalar.activation(
                                out=ex[:], in_=ee[:],
                                func=mybir.ActivationFunctionType.Exp)
                            for rr in range(Rbs):
                                oh = ohpool.tile([P, P], dt.float32, tag="oh")
                                nc.vector.tensor_scalar(
                                    out=oh[:], in0=iota_sb[:],
                                    scalar1=dstl_sb[:, c0b + rr:c0b + rr + 1],
                                    scalar2=ex[:, rr:rr + 1],
                                    op0=mybir.AluOpType.is_equal,
                                    op1=mybir.AluOpType.mult,
                                )
                                nc.tensor.matmul(
                                    out=pacc[:], lhsT=oh[:],
                                    rhs=tv[:, rb0 + rr, 0:1 + H],
                                    start=(done == 0),
                                    stop=(done == nch - 1),
                                )
                                done += 1
                        den = epipool.tile([P, 1], dt.float32, tag="den")
                        nc.vector.tensor_scalar_add(out=den[:],
                                                    in0=pacc[:, 0:1],
                                                    scalar1=1e-30)
                        rec = epipool.tile([P, 1], dt.float32, tag="rec")
                        nc.vector.reciprocal(out=rec[:], in_=den[:])
                        h = epipool.tile([P, H], dt.float32, tag="h")
                        nc.vector.tensor_scalar_mul(out=h[:], in0=pacc[:, 1:],
                                                    scalar1=rec[:])
                        nc.vector.tensor_tensor(out=h[:], in0=h[:],
                                                in1=bias_sb[:],
                                                op=mybir.AluOpType.add)
                        if relu:
                            nc.scalar.activation(
                                out=h[:], in_=h[:],
                                func=mybir.ActivationFunctionType.Relu)
                        out_writer(b, h)

            def l1_writer(b, h):
                pt = psumt.tile([H, P], dt.float32, tag="pt")
                nc.tensor.transpose(out=pt[:], in_=h[:], identity=ident[:])
                hT = prod.tile([H, P], dt.float32, tag="hT")
                nc.vector.tensor_copy(out=hT[:], in_=pt[:])
                pmm2 = psumt.tile([P, TW], dt.float32, tag="pmm")
                nc.tensor.matmul(out=pmm2[:], lhsT=hT[:], rhs=W2a_sb[:],
                                 start=True, stop=True)
                f2 = prod.tile([P, TW], dt.float32, tag="fsb")
                nc.vector.tensor_copy(out=f2[:, 1:], in_=pmm2[:, 1:])
                nc.vector.memset(f2[:, 0:1], 1.0)
                nc.sync.dma_start(out=feat2_s[b * P:(b + 1) * P, :],
                                  in_=f2[:])

            edge_phase(feat1_f, feat1_s, b1r_sb, True, l1_writer)
            allgather(feat2_s, feat2_f)

            def l2_writer(b, h):
                rows = LB if b == NB - 1 else P
                nc.sync.dma_start(out=out_ext[b * P:b * P + rows, :],
                                  in_=h[:rows, :])

            edge_phase(feat2_f, feat2_s, b2r_sb, False, l2_writer)

    nc.compile()
    return nc


def _get_program(R_key, single=False):
    key = ("prog", R_key, single)
    if key not in _cache:
        _cache[key] = _build_program(R_key, single=single)
    return _cache[key]


def kernel(x, src, dst, W1, al1, ar1, b1, W2, al2, ar2, b2):
    from concourse.bass_utils import run_bass_kernel_spmd

    in_maps, R_key = _host_prep(x, src, dst, W1, al1, ar1, b1,
                                W2, al2, ar2, b2)
    nc = _get_program(R_key)
    res = run_bass_kernel_spmd(nc, in_maps, list(range(NCORES)))
    out = np.concatenate([res.results[c]["out"] for c in range(NCORES)],
                         axis=0)
    return out.astype(np.float32)


# revision 25
# speedup vs baseline: 2033.0881x; 1.0013x over previous
"""GAT (2-layer, single-head) Trainium2 Bass kernel, 8-core SPMD.

Strategy (edge/graph parallelism per the sharding hint):
  - Destination nodes are 1D-sharded: core c owns nodes [c*12500, (c+1)*12500).
  - Edges are routed to the core owning their destination (host bucketing by
    dst), grouped into 128-node dst blocks; per block, edges are bucketed by
    source segment (32k node ranges) so gathers can use int16 indices.
  - Each core computes its slice of the per-node feature table
    feat = [1 | x@W | x@W@al | x@W@ar | pad] ([12544, 64] f32, 256B rows)
    and the full table is AllGathered so every core can gather src rows.
  - Edge phase per dst block: one dma_gather per (block-group, segment) pulls
    feat[src] rows; one dma_gather per block-group pulls er[dst] (from the
    core-local slice); attention logits e = leaky_relu(el[src] + er[dst]),
    ex = exp(e) (softmax shift-invariance: max-subtraction dropped; logits
    are O(1) so fp32 exp is safe); a one-hot segment-reduction matmul per
    128-edge chunk: onehot_ex = (iota == dst_local) * ex built in a single
    two-op tensor_scalar, then PSUM-accumulated out = onehot_ex^T @ [1|feat],
    giving softmax denominator (col 0) and numerator in one chain.
  - out_block = numerator / denom + bias (+ relu between layers); layer-2
    table is produced inline per block (PE transpose + matmul), AllGathered,
    and the edge phase repeats; each core writes its [12500, 32] out slice.
"""

import numpy as np

N = 100000
E = 1600000
F = 128
H = 32
NCORES = 8
NPC = N // NCORES          # nodes per core
P = 128
NB = (NPC + P - 1) // P    # dst blocks per core (98; last block 84 rows)
LB = NPC - (NB - 1) * P    # rows in last block
NPCP = NB * P              # padded nodes per core (table rows per core)
TROWS = NCORES * NPCP      # full table rows
TW = 64                    # table row: [1, feat(32), el, er, pad...] = 256B
ELCOL = 1 + H              # 33
ERCOL = 2 + H              # 34
SEG = 32768                # src segment size (int16 gather indices)
NSEG = (TROWS + SEG - 1) // SEG
GB = 1                     # dst blocks per gather group
NGB = (NB + GB - 1) // GB
CAPC = 8                   # max chunks (1024 indices) per dma_gather op

_cache = {}


def _plan(R):
    """Chunk-column layout shared by host prep and program build.

    R: [NB, NSEG] chunks per (block, segment).
    Returns (CH, groups) where groups[g] = (cg0, Rg, feat_ops, blocks);
    feat_ops[s] = (c0, R_gs); blocks[b] = list of (s, c0, Rbs).
    """
    groups = []
    c = 0
    for g in range(NGB):
        bs = list(range(g * GB, min((g + 1) * GB, NB)))
        cg0 = c
        feat_ops = []
        blocks = {b: [] for b in bs}
        for s in range(NSEG):
            c0 = c
            for b in bs:
                blocks[b].append((s, c, int(R[b, s])))
                c += int(R[b, s])
            feat_ops.append((c0, c - c0))
        groups.append((cg0, c - cg0, feat_ops, blocks))
    return c, groups


def _wrap16(i_op):
    """dma_gather index layout: op-local index i -> (row, col16)."""
    return i_op % 16, i_op // 16


def _host_prep(x, src, dst, W1, al1, ar1, b1, W2, al2, ar2, b2):
    f32, i16 = np.float32, np.int16
    src = np.asarray(src).astype(np.int64)
    dst = np.asarray(dst).astype(np.int64)

    core = dst // NPC
    r = dst % NPC
    b = r // P
    dl = (r % P).astype(f32)
    trow_src = (src // NPC) * NPCP + (src % NPC)
    seg = trow_src // SEG
    bgid = core * NB + b

    order = np.lexsort((seg, bgid))
    s_src_trow = trow_src[order]
    s_seg = seg[order]
    s_bgid = bgid[order]
    s_dl = dl[order]
    s_b = b[order]

    key = s_bgid * NSEG + s_seg
    counts = np.bincount(key, minlength=NCORES * NB * NSEG)
    counts3 = counts.reshape(NCORES, NB, NSEG)
    R = -(-counts3.max(axis=0) // P)          # [NB, NSEG] chunks (may be 0)
    R = np.maximum(R, 1)
    CH, groups = _plan(R)

    # per-edge rank within its (core, block, seg) run
    seg_start = np.concatenate([[0], np.cumsum(counts)])[:-1]
    rank = np.arange(len(order), dtype=np.int64) - seg_start[key]

    # chunk column of each (block, seg): c0 table
    c0_tab = np.zeros((NB, NSEG), np.int64)
    for g, (cg0, Rg, feat_ops, blocks) in enumerate(groups):
        for bb, lst in blocks.items():
            for (s, c0, _Rbs) in lst:
                c0_tab[bb, s] = c0
    cg0_of_b = np.zeros(NB, np.int64)
    for g, (cg0, Rg, feat_ops, blocks) in enumerate(groups):
        for bb in blocks:
            cg0_of_b[bb] = cg0

    col = c0_tab[s_b, s_seg] + rank // P
    p = rank % P

    dstl = np.full((NCORES, P, CH), 200.0, f32)
    flat = s_bgid // NB * (P * CH) + p * CH + col
    dstl.reshape(-1)[flat] = s_dl

    # feat gather indices: [16, CH*8] wrapped, relative to segment base
    fidx = np.zeros((NCORES, 16, CH * 8), i16)
    i_op = (col - c0_tab[s_b, s_seg]) * P + p
    row16, col16 = _wrap16(i_op)
    abscol = c0_tab[s_b, s_seg] * 8 + col16
    fflat = (s_bgid // NB) * (16 * CH * 8) + row16 * (CH * 8) + abscol
    fidx.reshape(-1)[fflat] = (s_src_trow - s_seg * SEG).astype(i16)

    # er gather indices: relative to group window (GB*128 rows)
    eidx = np.zeros((NCORES, 16, CH * 8), i16)
    i_op = (col - cg0_of_b[s_b]) * P + p
    row16, col16 = _wrap16(i_op)
    abscol = cg0_of_b[s_b] * 8 + col16
    eflat = (s_bgid // NB) * (16 * CH * 8) + row16 * (CH * 8) + abscol
    er_local = (s_b - (s_b // GB) * GB) * P + s_dl.astype(np.int64)
    eidx.reshape(-1)[eflat] = er_local.astype(i16)

    fidx = np.tile(fidx, (1, 8, 1))
    eidx = np.tile(eidx, (1, 8, 1))

    def aug(W, al, ar):
        Wa = np.zeros((W.shape[0], TW), f32)
        Wa[:, 1:1 + H] = W
        Wa[:, ELCOL] = W @ al
        Wa[:, ERCOL] = W @ ar
        return Wa

    W1a = aug(np.asarray(W1, f32), np.asarray(al1, f32), np.asarray(ar1, f32))
    W2a = aug(np.asarray(W2, f32), np.asarray(al2, f32), np.asarray(ar2, f32))
    b1r = np.tile(np.asarray(b1, f32)[None, :], (P, 1))
    b2r = np.tile(np.asarray(b2, f32)[None, :], (P, 1))
    iota = np.tile(np.arange(P, dtype=f32)[None, :], (P, 1))

    x = np.asarray(x, f32)
    xsT = np.zeros((NCORES, F, NPCP), f32)
    for cc in range(NCORES):
        xsT[cc, :, :NPC] = x[cc * NPC:(cc + 1) * NPC].T

    in_maps = []
    for cc in range(NCORES):
        in_maps.append({
            "xsT": xsT[cc],
            "W1a": W1a, "W2a": W2a, "b1r": b1r, "b2r": b2r, "iota": iota,
            "fidx": fidx[cc], "eidx": eidx[cc], "dstl": dstl[cc],
        })
    return in_maps, tuple(int(v) for v in R.reshape(-1))


def _build_program(R_key, single=False):
    import concourse.bacc as bacc
    import concourse.mybir as mybir
    import concourse.tile as tile
    from concourse.masks import make_identity

    dt = mybir.dt
    R = np.asarray(R_key, np.int64).reshape(NB, NSEG)
    CH, groups = _plan(R)
    ncores = 1 if single else NCORES

    nc = bacc.Bacc("TRN2", target_bir_lowering=False, debug=False,
                   num_devices=ncores, num_swdge_queues=4)

    xsT = nc.dram_tensor("xsT", [F, NPCP], dt.float32, kind="ExternalInput")
    W1a = nc.dram_tensor("W1a", [F, TW], dt.float32, kind="ExternalInput")
    W2a = nc.dram_tensor("W2a", [H, TW], dt.float32, kind="ExternalInput")
    b1r = nc.dram_tensor("b1r", [P, H], dt.float32, kind="ExternalInput")
    b2r = nc.dram_tensor("b2r", [P, H], dt.float32, kind="ExternalInput")
    iota = nc.dram_tensor("iota", [P, P], dt.float32, kind="ExternalInput")
    fidx = nc.dram_tensor("fidx", [P, CH * 8], dt.int16, kind="ExternalInput")
    eidx = nc.dram_tensor("eidx", [P, CH * 8], dt.int16, kind="ExternalInput")
    dstl = nc.dram_tensor("dstl", [P, CH], dt.float32, kind="ExternalInput")
    out_ext = nc.dram_tensor("out", [NPC, H], dt.float32, kind="ExternalOutput")

    qn_state = [0]

    def qn():
        qn_state[0] = (qn_state[0] + 1) % 4
        return qn_state[0]

    with tile.TileContext(nc) as tc:
        with (
            tc.tile_pool(name="const", bufs=1) as const,
            tc.tile_pool(name="prod", bufs=4) as prod,
            tc.tile_pool(name="gath", bufs=16) as gpool,
            tc.tile_pool(name="erg", bufs=8) as erpool,
            tc.tile_pool(name="edge", bufs=6) as epool,
            tc.tile_pool(name="oh", bufs=12) as ohpool,
            tc.tile_pool(name="epi", bufs=4) as epipool,
            tc.tile_pool(name="ps", bufs=4, space="PSUM") as psum,
            tc.tile_pool(name="pst", bufs=2, space="PSUM") as psumt,
            tc.tile_pool(name="dram", bufs=1, space="DRAM") as dram,
        ):
            iota_sb = const.tile([P, P], dt.float32)
            nc.sync.dma_start(out=iota_sb[:], in_=iota[:])
            W1a_sb = const.tile([F, TW], dt.float32)
            nc.sync.dma_start(out=W1a_sb[:], in_=W1a[:])
            W2a_sb = const.tile([H, TW], dt.float32)
            nc.sync.dma_start(out=W2a_sb[:], in_=W2a[:])
            b1r_sb = const.tile([P, H], dt.float32)
            nc.sync.dma_start(out=b1r_sb[:], in_=b1r[:])
            b2r_sb = const.tile([P, H], dt.float32)
            nc.sync.dma_start(out=b2r_sb[:], in_=b2r[:])
            fidx_sb = const.tile([P, CH * 8], dt.int16)
            nc.sync.dma_start(out=fidx_sb[:], in_=fidx[:])
            eidx_sb = const.tile([P, CH * 8], dt.int16)
            nc.sync.dma_start(out=eidx_sb[:], in_=eidx[:])
            dstl_sb = const.tile([P, CH], dt.float32)
            nc.sync.dma_start(out=dstl_sb[:], in_=dstl[:])
            ident = const.tile([P, P], dt.float32)
            make_identity(nc, ident[:])

            feat1_s = dram.tile([NPCP, TW], dt.float32)
            feat1_f = dram.tile([TROWS, TW], dt.float32, addr_space="Shared")
            feat2_s = dram.tile([NPCP, TW], dt.float32)
            feat2_f = dram.tile([TROWS, TW], dt.float32, addr_space="Shared")

            # ---- layer-1 table production ----
            for b in range(NB):
                xt = prod.tile([F, P], dt.float32, tag="xt")
                nc.sync.dma_start(out=xt[:], in_=xsT[:, b * P:(b + 1) * P])
                pmm = psumt.tile([P, TW], dt.float32, tag="pmm")
                nc.tensor.matmul(out=pmm[:], lhsT=xt[:], rhs=W1a_sb[:],
                                 start=True, stop=True)
                fsb = prod.tile([P, TW], dt.float32, tag="fsb")
                nc.vector.tensor_copy(out=fsb[:, 1:], in_=pmm[:, 1:])
                nc.vector.memset(fsb[:, 0:1], 1.0)
                nc.sync.dma_start(out=feat1_s[b * P:(b + 1) * P, :],
                                  in_=fsb[:])

            def allgather(src_t, dst_t):
                if single:
                    nc.sync.dma_start(out=dst_t[0:NPCP, :], in_=src_t[:])
                else:
                    nc.gpsimd.collective_compute(
                        "AllGather", mybir.AluOpType.bypass,
                        replica_groups=[list(range(NCORES))],
                        ins=[src_t[:]], outs=[dst_t[:]],
                    )

            allgather(feat1_s, feat1_f)

            # ---- edge phase ----
            def edge_phase(feat_f, feat_s, bias_sb, relu, out_writer):
                def emit_gather(tt, in_ap, idx_sb, c0, R):
                    # ucode caps one dma_gather at 1024 indices (8 chunks)
                    tv = tt[:].rearrange("p (r e) -> p r e", e=TW)
                    for off in range(0, R, CAPC):
                        take = min(CAPC, R - off)
                        nc.gpsimd.dma_gather(
                            out_ap=tv[:, off:off + take, :],
                            in_ap=in_ap,
                            idxs_ap=idx_sb[:, (c0 + off) * 8:
                                           (c0 + off + take) * 8],
                            num_idxs=take * P, num_idxs_reg=take * P,
                            elem_size=TW, queue_num=qn(),
                        )

                for g, (cg0, Rg, feat_ops, blocks) in enumerate(groups):
                    erg = erpool.tile([P, Rg * TW], dt.float32, tag="erg")
                    emit_gather(
                        erg,
                        feat_s[g * GB * P:(g * GB + len(blocks)) * P, :],
                        eidx_sb, cg0, Rg)
                    tts = []
                    for s, (c0, Rgs) in enumerate(feat_ops):
                        tt = gpool.tile([P, Rgs * TW], dt.float32,
                                        tag=f"T{s}")
                        seg_lo = s * SEG
                        seg_hi = min(seg_lo + SEG, TROWS)
                        emit_gather(tt, feat_f[seg_lo:seg_hi, :],
                                    fidx_sb, c0, Rgs)
                        tts.append((tt, c0))
                    for b in sorted(blocks):
                        chunks = blocks[b]   # [(s, c0, Rbs)]
                        nch = sum(rr for (_s, _c, rr) in chunks)
                        pacc = psum.tile([P, 1 + H], dt.float32, tag="pacc")
                        done = 0
                        for (s, c0b, Rbs) in chunks:
                            tt, c0op = tts[s]
                            tv = tt[:].rearrange("p (r e) -> p r e", e=TW)
                            ev = erg[:].rearrange("p (r e) -> p r e", e=TW)
                            rb0 = c0b - c0op
                            re0 = c0b - cg0
                            ee = epool.tile([P, Rbs], dt.float32, tag="ee")
                            nc.vector.tensor_tensor(
                                out=ee[:], in0=tv[:, rb0:rb0 + Rbs, ELCOL],
                                in1=ev[:, re0:re0 + Rbs, ERCOL],
                                op=mybir.AluOpType.add)
                            et = epool.tile([P, Rbs], dt.float32, tag="et")
                            nc.vector.tensor_scalar_mul(out=et[:], in0=ee[:],
                                                        scalar1=0.2)
                            nc.vector.tensor_tensor(
                                out=ee[:], in0=ee[:], in1=et[:],
                                op=mybir.AluOpType.max)
                            ex = epool.tile([P, Rbs], dt.float32, tag="ex")
                            nc.scalar.activation(
                                out=ex[:], in_=ee[:],
                                func=mybir.ActivationFunctionType.Exp)
                            for rr in range(Rbs):
                                oh = ohpool.tile([P, P], dt.float32, tag="oh")
                                nc.vector.tensor_scalar(
                                    out=oh[:], in0=iota_sb[:],
                                    scalar1=dstl_sb[:, c0b + rr:c0b + rr + 1],
                                    scalar2=ex[:, rr:rr + 1],
                                    op0=mybir.AluOpType.is_equal,
                                    op1=mybir.AluOpType.mult,
                                )
                                nc.tensor.matmul(
                                    out=pacc[:], lhsT=oh[:],
                                    rhs=tv[:, rb0 + rr, 0:1 + H],
                                    start=(done == 0),
                                    stop=(done == nch - 1),
                                )
                                done += 1
                        den = epipool.tile([P, 1], dt.float32, tag="den")
                        nc.vector.tensor_scalar_add(out=den[:],
                                                    in0=pacc[:, 0:1],
                                                    scalar1=1e-30)
                        rec = epipool.tile([P, 1], dt.float32, tag="rec")
                        nc.vector.reciprocal(out=rec[:], in_=den[:])
                        h = epipool.tile([P, H], dt.float32, tag="h")
                        nc.vector.tensor_scalar_mul(out=h[:], in0=pacc[:, 1:],
                                                    scalar1=rec[:])
                        nc.vector.tensor_tensor(out=h[:], in0=h[:],
                                                in1=bias_sb[:],
                                                op=mybir.AluOpType.add)
                        if relu:
                            nc.scalar.activation(
                                out=h[:], in_=h[:],
                                func=mybir.ActivationFunctionType.Relu)
                        out_writer(b, h)

            def l1_writer(b, h):
                pt = psumt.tile([H, P], dt.float32, tag="pt")
                nc.tensor.transpose(out=pt[:], in_=h[:], identity=ident[:])
                hT = prod.tile([H, P], dt.float32, tag="hT")
                nc.vector.tensor_copy(out=hT[:], in_=pt[:])
                pmm2 = psumt.tile([P, TW], dt.float32, tag="pmm")
                nc.tensor.matmul(out=pmm2[:], lhsT=hT[:], rhs=W2a_sb[:],
                                 start=True, stop=True)
                f2 = prod.tile([P, TW], dt.float32, tag="fsb")
                nc.vector.tensor_copy(out=f2[:, 1:], in_=pmm2[:, 1:])
                nc.vector.memset(f2[:, 0:1], 1.0)
                nc.sync.dma_start(out=feat2_s[b * P:(b + 1) * P, :],
                                  in_=f2[:])

            edge_phase(feat1_f, feat1_s, b1r_sb, True, l1_writer)
            allgather(feat2_s, feat2_f)

            def l2_writer(b, h):
                rows = LB if b == NB - 1 else P
                nc.sync.dma_start(out=out_ext[b * P:b * P + rows, :],
                                  in_=h[:rows, :])

            edge_phase(feat2_f, feat2_s, b2r_sb, False, l2_writer)

    nc.compile()
    return nc


def _get_program(R_key, single=False):
    key = ("prog", R_key, single)
    if key not in _cache:
        _cache[key] = _build_program(R_key, single=single)
    return _cache[key]


def kernel(x, src, dst, W1, al1, ar1, b1, W2, al2, ar2, b2):
    from concourse.bass_utils import run_bass_kernel_spmd

    in_maps, R_key = _host_prep(x, src, dst, W1, al1, ar1, b1,
                                W2, al2, ar2, b2)
    nc = _get_program(R_key)
    res = run_bass_kernel_spmd(nc, in_maps, list(range(NCORES)))
    out = np.concatenate([res.results[c]["out"] for c in range(NCORES)],
                         axis=0)
    return out.astype(np.float32)
